# revision 11
# baseline (speedup 1.0000x reference)
"""Trainium2 Bass kernel for nn_BackboneSolver (GNN frame message-passing).

Factored algorithm (numpy-validated, rel-err ~1e-6 vs reference):
  pairwise state T_p^(k) = TFm[i] @ T_p0 @ TDm[j] is kept factored via
  per-node cumulative transforms (reference's transpose-as-inverse
  convention), so per iteration each edge needs only
    B1_e    = G'[j_e] @ Op0_e^T            (G' = O_cur @ ODm^T, node level)
    v_e     = W[n] @ (tp0_e + Op0_e @ tDm[j_e])        (iter 1: v = tp0)
    Osum[n] = ((sum_k w B1) @ OFm^T[n]) * invS
    trans[n]= (sum_k w t[j] - sum_k (w B1) v - (sum_k w B1) a[n]) * invS
  O_new = Kabsch(Osum) via 3-sweep cyclic Jacobi of Osum^T Osum (det-safe
  column sort, u3 = u1 x u2).

Sharding: B=16 over 8 cores; each core runs its 2 examples sequentially in
one NEFF.  Gathers via gpsimd dma_gather from an HBM table (64-fp32 rows,
[G' 9 | t 3 | tDm 3 | pad]) rebuilt per iteration.

Layouts (per example): edge planes [128, N/128*K] (p=n%128, f=(n//128)*K+k),
node planes [128, N/128] (p=n%128, f=n//128).
"""
import contextlib
import numpy as np

import concourse.bacc as bacc
import concourse.bass as bass
import concourse.mybir as mybir
from concourse.tile import TileContext
from concourse.bass_utils import run_bass_kernel_spmd

F32 = mybir.dt.float32
I16 = mybir.dt.int16
AL = mybir.AluOpType
AF = mybir.ActivationFunctionType

B_FULL, N_FULL, K_FULL = 16, 4096, 64
N_CORES = 8
NUM_ITERS = 3

GATHER_CHUNK = 8192      # idxs per dma_gather (hw-validated)
ROW = 64                 # fp32 row slots (256B DMA-gather granularity)
RG, RT, RD = 0, 9, 12    # row slots: G' 3x3, t_node, tDm


def _bc(ap2d, k):
    """[128, F] -> [128, F, k] broadcast AP (innermost step 0)."""
    return bass.AP(ap2d.tensor, ap2d.offset, list(ap2d.ap) + [[0, k]])


def build_nc(N=N_FULL, K=K_FULL, n_ex=2, sc_cols=128, n_iters=NUM_ITERS, skip_gather=False):
    EPK = N // 128 * K
    NPC = N // 128
    n_sc = EPK // sc_cols
    g_per_sc = max(1, sc_cols * 128 // GATHER_CHUNK)
    chunk = min(GATHER_CHUNK, sc_cols * 128)
    cpg = chunk // 128                       # aos cols per gather
    na = sc_cols // K                        # node cols per super-chunk

    nc = bacc.Bacc("TRN2")
    res_in, idx_in, node_in, out_t, tables = [], [], [], [], []
    for e in range(n_ex):
        res_in.append(nc.dram_tensor(f"res{e}", [13, 128, EPK], F32,
                                     kind="ExternalInput"))
        idx_in.append(nc.dram_tensor(f"idx{e}", [128, EPK * 8], I16,
                                     kind="ExternalInput"))
        node_in.append(nc.dram_tensor(f"node{e}", [12, 128, NPC], F32,
                                      kind="ExternalInput"))
        out_t.append(nc.dram_tensor(f"out{e}", [12, 128, NPC], F32,
                                    kind="ExternalOutput"))
        tables.append(nc.dram_tensor(f"table{e}", [N, ROW], F32,
                                     kind="Internal"))

    with TileContext(nc) as tc, contextlib.ExitStack() as ctx:
        aosp = ctx.enter_context(tc.tile_pool(name="aos", bufs=2))
        strp = ctx.enter_context(tc.tile_pool(name="stream", bufs=2))
        idxp = ctx.enter_context(tc.tile_pool(name="idx", bufs=2))
        tmpp = ctx.enter_context(tc.tile_pool(name="etmp", bufs=2))
        nodp = ctx.enter_context(tc.tile_pool(name="node", bufs=1))
        stgp = ctx.enter_context(tc.tile_pool(name="stage", bufs=2))

        V = nc.vector
        SC = nc.scalar

        def nt(name):
            return nodp.tile([128, NPC], F32, tag=name, name=name)

        def ntu8(name):
            return nodp.tile([128, NPC], mybir.dt.uint8, tag=name, name=name)

        def et(name):
            return tmpp.tile([128, sc_cols], F32, tag=name, name=name)

        def mul(o, a, b):
            V.tensor_tensor(_a(o), _a(a), _a(b), AL.mult)

        def add(o, a, b):
            V.tensor_tensor(_a(o), _a(a), _a(b), AL.add)

        def sub(o, a, b):
            V.tensor_tensor(_a(o), _a(a), _a(b), AL.subtract)

        def _a(x):
            return x if isinstance(x, bass.AP) else x[:]

        def tsmul(o, a, c):
            V.tensor_single_scalar(_a(o), _a(a), float(c), op=AL.mult)

        def dot3(out, xs, ys, t1):
            """out = sum_i xs[i]*ys[i] (3 terms) using temp t1"""
            mul(t1, xs[0], ys[0])
            mul(out, xs[1], ys[1])
            add(out, out, t1)
            mul(t1, xs[2], ys[2])
            add(out, out, t1)

        zero_n = nt("zero")
        V.memset(zero_n[:], 0.0)
        eps_n = nt("epsn")
        V.memset(eps_n[:], 1e-30)
        eps20 = nodp.tile([128, 1], F32, tag="eps20", name="eps20")
        V.memset(eps20[:], 1e-20)

        for e in range(n_ex):
            # ---- node state (persistent tags per example) ----
            Ocur = [nt(f"oc{e}_{c}") for c in range(9)]
            tcur = [nt(f"tc{e}_{c}") for c in range(3)]
            for c in range(9):
                nc.gpsimd.dma_start(Ocur[c][:], node_in[e][c])
            for c in range(3):
                nc.gpsimd.dma_start(tcur[c][:], node_in[e][9 + c])
            ODm = [nt(f"odm{e}_{c}") for c in range(9)]
            tDm = [nt(f"tdm{e}_{c}") for c in range(3)]
            OFm = [nt(f"ofm{e}_{c}") for c in range(9)]
            tFm = [nt(f"tfm{e}_{c}") for c in range(3)]
            Wn = [nt(f"wn{e}_{c}") for c in range(9)]
            an = [nt(f"an{e}_{c}") for c in range(3)]

            # ---- invS = 1/sum_k w ----
            invS = nt(f"invs{e}")
            wfull = stgp.tile([128, EPK], F32, tag="wfull", name="wfull")
            nc.gpsimd.dma_start(wfull[:], res_in[e][9])
            ssum = nt("ssum")
            V.tensor_reduce(ssum[:],
                            wfull[:].rearrange("p (a k) -> p a k", k=K),
                            axis=mybir.AxisListType.X, op=AL.add)
            V.reciprocal(invS[:], ssum[:])

            for it in range(n_iters):
                # ---------- table build ----------
                stage = stgp.tile([128, NPC, ROW], F32, tag="stage", name="stage")
                if it == 0:
                    Gp = Ocur
                else:
                    Gp = []
                    for i in range(3):
                        for j in range(3):
                            acc = nt(f"gp{3 * i + j}")
                            dot3(acc, [Ocur[3 * i + l] for l in range(3)],
                                 [ODm[3 * j + l] for l in range(3)],
                                 nt("t1g"))
                            Gp.append(acc)
                for c in range(9):
                    V.tensor_copy(stage[:, :, RG + c], Gp[c][:])
                for c in range(3):
                    V.tensor_copy(stage[:, :, RT + c], tcur[c][:])
                    V.tensor_copy(stage[:, :, RD + c],
                                  (tDm[c] if it > 0 else zero_n)[:])
                nc.gpsimd.dma_start(
                    tables[e][:].rearrange("(a p) c -> p a c", p=128), stage[:])

                # ---------- edge phase ----------
                OsumP = [nt(f"osp{c}") for c in range(9)]
                tsumP = [nt(f"tsp{c}") for c in range(3)]

                for sc in range(n_sc):
                    c0 = sc * sc_cols
                    a0 = c0 // K
                    # stream resident slice: [13, 128, sc_cols] -> [128,13,sc]
                    st = strp.tile([128, 13, sc_cols], F32, tag="st", name="st")
                    nc.gpsimd.dma_start(
                        st[:],
                        res_in[e][:, :, c0:c0 + sc_cols].rearrange(
                            "c p f -> p c f"))

                    aos = aosp.tile([128, sc_cols, ROW], F32, tag="aos", name="aos")
                    for g in range(g_per_sc):
                        i0 = (sc * g_per_sc + g) * (chunk // 16)
                        idxt = idxp.tile([128, chunk // 16], I16, tag="idxt", name="idxt")
                        nc.gpsimd.dma_start(
                            idxt[:], idx_in[e][:, i0:i0 + chunk // 16])
                        if not skip_gather:
                            nc.gpsimd.dma_gather(
                                aos[:, g * cpg:(g + 1) * cpg, :], tables[e][:],
                                idxt[:], chunk, chunk, ROW, single_packet=False)
                        else:
                            h = cpg // 2
                            for hh in range(2):
                                nc.gpsimd.dma_start(
                                    aos[:, g * cpg + hh * h:
                                        g * cpg + (hh + 1) * h, :],
                                    res_in[e][hh][:, 0:h * ROW].rearrange(
                                        "p (a k) -> p a k", k=ROW))

                    def g_(c):
                        return aos[:, :, c]

                    def r_(c):          # resident comp slice [128, sc_cols]
                        return st[:, c, :]

                    def r3(c):
                        return st[:, c, :].rearrange("p (a k) -> p a k", k=K)

                    def nb(pl):         # node window broadcast over k (3D)
                        return _bc(pl[:, a0:a0 + na], K)

                    def e3(t):          # temp tile as 3D view
                        return t[:].rearrange("p (a k) -> p a k", k=K)

                    # ---- v (iters >= 2): v = W @ (tp0 + Op0 @ tD[j]) ----
                    vpl = []
                    if it > 0:
                        up = []
                        for l in range(3):
                            u = et(f"u{l}")
                            t1 = et("t1e")
                            # (Op0 @ tD)_l = sum_j Op0T[j,l] tD_j
                            mul(u, g_(RD + 0), r_(0 * 3 + l))
                            mul(t1, g_(RD + 1), r_(1 * 3 + l))
                            add(u, u, t1)
                            mul(t1, g_(RD + 2), r_(2 * 3 + l))
                            add(u, u, t1)
                            add(u, u, r_(10 + l))       # + tp0
                            up.append(u)
                        for l in range(3):
                            v = et(f"v{l}")
                            t1 = et("t1e")
                            mul(e3(v), nb(Wn[3 * l + 0]), e3(up[0]))
                            mul(e3(t1), nb(Wn[3 * l + 1]), e3(up[1]))
                            add(v, v, t1)
                            mul(e3(t1), nb(Wn[3 * l + 2]), e3(up[2]))
                            add(v, v, t1)
                            vpl.append(v)
                    else:
                        vpl = [r_(10 + l) for l in range(3)]

                    # ---- tvec = w*t_j ; B1/wB1 + reductions ----
                    tvec = []
                    for i in range(3):
                        tv = et(f"tv{i}")
                        mul(tv, g_(RT + i), r_(9))
                        tvec.append(tv)
                    for i in range(3):
                        for l in range(3):
                            b1 = et("b1")
                            t1 = et("t1e")
                            mul(b1, g_(RG + 3 * i + 0), r_(0 * 3 + l))
                            mul(t1, g_(RG + 3 * i + 1), r_(1 * 3 + l))
                            add(b1, b1, t1)
                            mul(t1, g_(RG + 3 * i + 2), r_(2 * 3 + l))
                            add(b1, b1, t1)
                            mul(b1, b1, r_(9))          # * w
                            V.tensor_reduce(
                                OsumP[3 * i + l][:, a0:a0 + na], e3(b1),
                                axis=mybir.AxisListType.X, op=AL.add)
                            mul(t1, b1, vpl[l])
                            sub(tvec[i], tvec[i], t1)
                    for i in range(3):
                        V.tensor_reduce(
                            tsumP[i][:, a0:a0 + na], e3(tvec[i]),
                            axis=mybir.AxisListType.X, op=AL.add)

                # ---------- node phase ----------
                t1n = nt("t1n")
                if it > 0:
                    for i in range(3):
                        for l in range(3):
                            mul(t1n, OsumP[3 * i + l], an[l])
                            sub(tsumP[i], tsumP[i], t1n)
                    Os = []
                    for i in range(3):
                        for j in range(3):
                            acc = nt(f"os{3 * i + j}")
                            dot3(acc, [OsumP[3 * i + l] for l in range(3)],
                                 [OFm[3 * j + l] for l in range(3)], t1n)
                            Os.append(acc)
                else:
                    Os = OsumP
                for c in range(9):
                    mul(Os[c], Os[c], invS)
                tnew = []
                for i in range(3):
                    tn = nt(f"tnew{i}")
                    mul(tn, tsumP[i], invS)
                    tnew.append(tn)

                Onew = _jacobi_kabsch(nc, Os, nt, ntu8, zero_n, eps_n, eps20,
                                      mul, add, sub, tsmul, dot3, V, SC)

                if it < n_iters - 1:
                    # OD = Ocur^T @ Onew  (OF = OD^T by relabel)
                    OD = []
                    for i in range(3):
                        for j in range(3):
                            acc = nt(f"od{3 * i + j}")
                            dot3(acc, [Ocur[3 * l + i] for l in range(3)],
                                 [Onew[3 * l + j] for l in range(3)], t1n)
                            OD.append(acc)
                    diff = []
                    for i in range(3):
                        d = nt(f"df{i}")
                        sub(d, tnew[i], tcur[i])
                        diff.append(d)
                    td, tf = [], []
                    for i in range(3):
                        acc = nt(f"td{i}")
                        dot3(acc, [Ocur[3 * l + i] for l in range(3)], diff,
                             t1n)
                        td.append(acc)
                    for i in range(3):
                        acc = nt(f"tf{i}")
                        dot3(acc, [Onew[3 * l + i] for l in range(3)], diff,
                             t1n)
                        tsmul(acc, acc, -1.0)
                        tf.append(acc)

                    if it == 0:
                        for c in range(9):
                            V.tensor_copy(ODm[c][:], OD[c][:])
                            i, j = divmod(c, 3)
                            V.tensor_copy(OFm[c][:], OD[3 * j + i][:])
                        for c in range(3):
                            V.tensor_copy(tDm[c][:], td[c][:])
                            V.tensor_copy(tFm[c][:], tf[c][:])
                        # W = OFm^T OFm (constant afterwards)
                        for i in range(3):
                            for j in range(3):
                                dot3(Wn[3 * i + j],
                                     [OFm[3 * l + i] for l in range(3)],
                                     [OFm[3 * l + j] for l in range(3)], t1n)
                    else:
                        # tDm += ODm @ td
                        for i in range(3):
                            tmp = nt("ntmp0")
                            dot3(tmp, [ODm[3 * i + l] for l in range(3)], td,
                                 t1n)
                            add(tDm[i], tDm[i], tmp)
                        # ODm = ODm @ OD
                        ODn = []
                        for i in range(3):
                            for j in range(3):
                                acc = nt(f"odn{3 * i + j}")
                                dot3(acc, [ODm[3 * i + l] for l in range(3)],
                                     [OD[3 * l + j] for l in range(3)], t1n)
                                ODn.append(acc)
                        for c in range(9):
                            V.tensor_copy(ODm[c][:], ODn[c][:])
                        # tFm = tf + OF @ tFm   (OF[i,j] = OD[j,i])
                        tFn = []
                        for i in range(3):
                            acc = nt(f"tfn{i}")
                            dot3(acc, [OD[3 * l + i] for l in range(3)], tFm,
                                 t1n)
                            add(acc, acc, tf[i])
                            tFn.append(acc)
                        for c in range(3):
                            V.tensor_copy(tFm[c][:], tFn[c][:])
                        # OFm = OF @ OFm
                        OFn = []
                        for i in range(3):
                            for j in range(3):
                                acc = nt(f"ofn{3 * i + j}")
                                dot3(acc, [OD[3 * l + i] for l in range(3)],
                                     [OFm[3 * l + j] for l in range(3)], t1n)
                                OFn.append(acc)
                        for c in range(9):
                            V.tensor_copy(OFm[c][:], OFn[c][:])
                    # a = OFm^T tFm
                    for i in range(3):
                        dot3(an[i], [OFm[3 * l + i] for l in range(3)], tFm,
                             t1n)

                # state <- new
                for c in range(9):
                    V.tensor_copy(Ocur[c][:], Onew[c][:])
                for c in range(3):
                    V.tensor_copy(tcur[c][:], tnew[c][:])

            ost = stgp.tile([128, 12, NPC], F32, tag="ost", name="ost")
            for c in range(9):
                V.tensor_copy(ost[:, c, :], Ocur[c][:])
            for c in range(3):
                V.tensor_copy(ost[:, 9 + c, :], tcur[c][:])
            nc.gpsimd.dma_start(out_t[e][:].rearrange("c p a -> p c a"),
                                ost[:])

    nc.compile()
    return nc


def _jacobi_kabsch(nc, A, nt, ntu8, zero_n, eps_n, eps20, mul, add, sub,
                   tsmul, dot3, V, SC):
    """3-sweep Jacobi on S = A^T A; returns R = V_j U^T (9 node planes)."""
    t1 = nt("jt1")
    Sp = {}
    for i in range(3):
        for j in range(i, 3):
            acc = nt(f"js{i}{j}")
            dot3(acc, [A[3 * l + i] for l in range(3)],
                 [A[3 * l + j] for l in range(3)], t1)
            Sp[(i, j)] = acc
    Vp = []
    for i in range(3):
        for j in range(3):
            v = nt(f"jv{i}{j}")
            V.memset(v[:], 1.0 if i == j else 0.0)
            Vp.append(v)

    def S_(i, j):
        return Sp[(i, j)] if i <= j else Sp[(j, i)]

    for sw in range(3):
        for (p, q) in ((0, 1), (0, 2), (1, 2)):
            r = 3 - p - q
            d1 = nt("jd1"); sub(d1, S_(q, q), S_(p, p))
            two = nt("jtwo"); tsmul(two, S_(p, q), 2.0)
            absv = nt("jabs"); SC.activation(absv[:], two[:], AF.Abs)
            mask = ntu8("jmask")
            V.tensor_single_scalar(mask[:], absv[:], 1e-30, op=AL.is_ge)
            safe = nt("jsafe"); V.select(safe[:], mask[:], two[:], eps_n[:])
            rec = nt("jrec"); V.reciprocal(rec[:], safe[:])
            tau = nt("jtau"); mul(tau, d1, rec)
            tau2 = nt("jtau2"); mul(tau2, tau, tau)
            rt = nt("jrt"); SC.activation(rt[:], tau2[:], AF.Sqrt, bias=1.0)
            ngt = nt("jngt"); tsmul(ngt, rt, -1.0)
            gem = ntu8("jgem")
            V.tensor_single_scalar(gem[:], tau[:], 0.0, op=AL.is_ge)
            cs = nt("jcs"); V.select(cs[:], gem[:], rt[:], ngt[:])
            den = nt("jden"); add(den, tau, cs)
            tv = nt("jtv"); V.reciprocal(tv[:], den[:])
            tz = nt("jtz"); V.select(tz[:], mask[:], tv[:], zero_n[:])
            t2 = nt("jt2"); mul(t2, tz, tz)
            c2 = nt("jc2"); SC.activation(c2[:], t2[:], AF.Sqrt, bias=1.0)
            cv = nt("jcv"); V.reciprocal(cv[:], c2[:])
            sv = nt("jsv"); mul(sv, tz, cv)
            # S update (into fresh planes, then rebind)
            tspq = nt("jtspq"); mul(tspq, tz, S_(p, q))
            npp = nt("jnpp"); sub(npp, S_(p, p), tspq)
            nqq = nt("jnqq"); add(nqq, S_(q, q), tspq)
            n1 = nt("jn1"); mul(n1, cv, S_(p, r) if p <= r else S_(r, p))
            spr = S_(p, r)
            sqr = S_(q, r)
            n2 = nt("jn2"); mul(n2, sv, sqr)
            npr = nt("jnpr"); sub(npr, n1, n2)
            n3 = nt("jn3"); mul(n3, sv, spr)
            n4 = nt("jn4"); mul(n4, cv, sqr)
            nqr = nt("jnqr"); add(nqr, n3, n4)
            V.tensor_copy(S_(p, p)[:], npp[:])
            V.tensor_copy(S_(q, q)[:], nqq[:])
            V.memset(S_(p, q)[:], 0.0)
            V.tensor_copy(S_(p, r)[:] if p <= r else S_(r, p)[:], npr[:])
            V.tensor_copy(S_(q, r)[:] if q <= r else S_(r, q)[:], nqr[:])
            # V column rotation
            for i in range(3):
                vp_, vq_ = Vp[3 * i + p], Vp[3 * i + q]
                a1 = nt("ja1"); mul(a1, cv, vp_)
                a2 = nt("ja2"); mul(a2, sv, vq_)
                a3 = nt("ja3"); mul(a3, sv, vp_)
                a4 = nt("ja4"); mul(a4, cv, vq_)
                sub(a1, a1, a2)
                add(a3, a3, a4)
                V.tensor_copy(vp_[:], a1[:])
                V.tensor_copy(vq_[:], a3[:])

    # B = A @ V
    Bp = []
    for i in range(3):
        for j in range(3):
            acc = nt(f"jb{i}{j}")
            dot3(acc, [A[3 * i + l] for l in range(3)],
                 [Vp[3 * l + j] for l in range(3)], t1)
            Bp.append(acc)
    nrm = []
    for j in range(3):
        acc = nt(f"jn{j}")
        dot3(acc, [Bp[3 * l + j] for l in range(3)],
             [Bp[3 * l + j] for l in range(3)], t1)
        nrm.append(acc)
    # det-preserving descending sort of (B, V) columns by nrm
    for (i, j) in ((0, 1), (0, 2), (1, 2)):
        msk = ntu8("jsmsk")
        V.tensor_tensor(msk[:], nrm[j][:], nrm[i][:], AL.is_gt)
        for M in (Bp, Vp):
            for row in range(3):
                ci, cj = M[3 * row + i], M[3 * row + j]
                ngi = nt("jsneg"); tsmul(ngi, ci, -1.0)
                ni_ = nt("jsni"); V.select(ni_[:], msk[:], cj[:], ci[:])
                nj_ = nt("jsnj"); V.select(nj_[:], msk[:], ngi[:], cj[:])
                V.tensor_copy(ci[:], ni_[:])
                V.tensor_copy(cj[:], nj_[:])
        ta = nt("jsk1"); V.select(ta[:], msk[:], nrm[j][:], nrm[i][:])
        tb = nt("jsk2"); V.select(tb[:], msk[:], nrm[i][:], nrm[j][:])
        V.tensor_copy(nrm[i][:], ta[:])
        V.tensor_copy(nrm[j][:], tb[:])
    # u1
    rs = nt("jrs"); SC.activation(rs[:], nrm[0][:], AF.Sqrt, bias=eps20[:])
    r1 = nt("jr1"); V.reciprocal(r1[:], rs[:])
    U = [None] * 9
    for i in range(3):
        u = nt(f"ju{i}0"); mul(u, Bp[3 * i + 0], r1)
        U[3 * i + 0] = u
    # Gram-Schmidt + u2
    d = nt("jgd")
    dot3(d, [U[3 * l + 0] for l in range(3)],
         [Bp[3 * l + 1] for l in range(3)], t1)
    b2 = []
    for i in range(3):
        x = nt(f"jb2{i}")
        mul(x, d, U[3 * i + 0])
        sub(x, Bp[3 * i + 1], x)
        b2.append(x)
    nn2 = nt("jnn2")
    dot3(nn2, b2, b2, t1)
    rs2 = nt("jrs2"); SC.activation(rs2[:], nn2[:], AF.Sqrt, bias=eps20[:])
    r2 = nt("jr2"); V.reciprocal(r2[:], rs2[:])
    for i in range(3):
        u = nt(f"ju{i}1"); mul(u, b2[i], r2)
        U[3 * i + 1] = u
    # u3 = u1 x u2
    cr = [(1, 2), (2, 0), (0, 1)]
    for i in range(3):
        a_, b_ = cr[i]
        x1 = nt("jx1"); mul(x1, U[3 * a_ + 0], U[3 * b_ + 1])
        x2 = nt("jx2"); mul(x2, U[3 * b_ + 0], U[3 * a_ + 1])
        u = nt(f"ju{i}2"); sub(u, x1, x2)
        U[3 * i + 2] = u
    # R = V @ U^T
    R = []
    for i in range(3):
        for j in range(3):
            acc = nt(f"jr{i}{j}")
            dot3(acc, [Vp[3 * i + l] for l in range(3)],
                 [U[3 * j + l] for l in range(3)], t1)
            R.append(acc)
    return R


# ---------------------------------------------------------------------------
# host-side packing (layout only, no arithmetic)
# ---------------------------------------------------------------------------

def _pack_example(t0, O0, tp, Op, w, topo, N, K):
    NPC, EPK = N // 128, N // 128 * K

    def eplane(x):
        return np.ascontiguousarray(
            x.reshape(NPC, 128, K).transpose(1, 0, 2).reshape(128, EPK))

    res = np.empty((13, 128, EPK), np.float32)
    for j in range(3):
        for l in range(3):
            res[3 * j + l] = eplane(Op[:, :, l, j])   # Op0^T[j,l]
    res[9] = eplane(w)
    for c in range(3):
        res[10 + c] = eplane(tp[:, :, c])

    node = np.empty((12, 128, NPC), np.float32)

    def nplane(x):
        return np.ascontiguousarray(x.reshape(NPC, 128).T)

    for i in range(3):
        for j in range(3):
            node[3 * i + j] = nplane(O0[:, i, j])
    for c in range(3):
        node[9 + c] = nplane(t0[:, c])

    M = topo.reshape(NPC, 128, K).transpose(1, 0, 2).reshape(128, EPK)
    lin = np.ascontiguousarray(M.T).reshape(-1).astype(np.int16)
    wrapped = lin.reshape(-1, 16).T                    # [16, S]
    idx = np.ascontiguousarray(np.tile(wrapped, (8, 1)))
    return res, idx, node


def _unpack_out(o, N):
    NPC = N // 128
    O = np.empty((N, 3, 3), np.float32)
    t = np.empty((N, 3), np.float32)
    for i in range(3):
        for j in range(3):
            O[:, i, j] = o[3 * i + j].T.reshape(-1)
    for c in range(3):
        t[:, c] = o[9 + c].T.reshape(-1)
    return t, O


_NC_CACHE = {}


def kernel(translations, orientations, pair_translations, pair_orientations,
           confidences, topology):
    B, N, K = confidences.shape
    n_ex = B // N_CORES
    key = (N, K, n_ex)
    if key not in _NC_CACHE:
        _NC_CACHE[key] = build_nc(N=N, K=K, n_ex=n_ex)
    nc = _NC_CACHE[key]

    in_maps = []
    for c in range(N_CORES):
        m = {}
        for e in range(n_ex):
            b = c * n_ex + e
            res, idx, node = _pack_example(
                np.asarray(translations[b], np.float32),
                np.asarray(orientations[b], np.float32),
                np.asarray(pair_translations[b], np.float32),
                np.asarray(pair_orientations[b], np.float32),
                np.asarray(confidences[b], np.float32),
                np.asarray(topology[b]), N, K)
            m[f"res{e}"], m[f"idx{e}"], m[f"node{e}"] = res, idx, node
        in_maps.append(m)

    results = run_bass_kernel_spmd(nc, in_maps, core_ids=list(range(N_CORES)))

    t_out = np.empty((B, N, 3), np.float32)
    O_out = np.empty((B, N, 3, 3), np.float32)
    for c in range(N_CORES):
        for e in range(n_ex):
            b = c * n_ex + e
            t, O = _unpack_out(results.results[c][f"out{e}"], N)
            t_out[b], O_out[b] = t, O
    return t_out, O_out
